# revision 3
# baseline (speedup 1.0000x reference)
"""HadamardQuantLinear Trainium2 kernel.

Computes, for x:[B,S,K] f32, weight:[M,K] f32, bias:[M] f32, H:[64,64] f32:
    xb = blockhad(x, H/8); wb = blockhad(w, H/8)            (64-blocks on K)
    q, s = per-tensor symmetric int8 quant of xb and wb
    out = (xq @ wq.T) * (sx*sw) + bias

Sharding: data-parallel over tokens (B*S) across 8 cores; weight/bias/H
replicated. Host marshalling only: token-sharding, pre-transposed layouts
(K on partitions), and a 128x128 block-diagonal copy of H (pure data
movement; the 1/8 normalization folds exactly into the quant scales as a
power of two, so device-side math is bit-equivalent to the reference).

Device per core:
  pass1: rotate (PE, fp32) streaming xT/wT k-chunks, track |.|max     -> scales
         (x's max is AllReduce'd across the 8 cores; max is exact)
  pass2: rotate again, quantize with round-half-even (magic-number) to bf16
  main:  bf16 GEMM (values are small ints -> exact), fused dequant+bias
"""

import os

import numpy as np

NCORES = 8
P = 128
BLOCK = 64
MAGIC = 12582912.0  # 1.5 * 2**23: adding forces RNE to integer in fp32

_CACHE = {}


def _emit(tc, nc, xT, wT, bias, h2, out, tok, k, m, ncores, rot_dtype):
    import concourse.bass as bass
    import concourse.mybir as mybir
    from concourse.bass_isa import ReduceOp
    from contextlib import ExitStack

    f32 = mybir.dt.float32
    bf16 = mybir.dt.bfloat16

    kc_n = k // P            # number of 128-row k-chunks
    nfree = 512              # psum free dim
    ct_x = tok // nfree      # rotation column tiles per chunk (x)
    ct_w = m // nfree        # rotation column tiles per chunk (w)
    nt_n = tok // P          # output token tiles
    mt_n = m // nfree        # output feature tiles

    ctx = ExitStack()
    with ctx:
        singles = ctx.enter_context(tc.tile_pool(name="singles", bufs=1))
        stream = ctx.enter_context(tc.tile_pool(name="stream", bufs=4))
        pr = ctx.enter_context(tc.tile_pool(name="pr", bufs=4, space="PSUM"))
        pm = ctx.enter_context(tc.tile_pool(name="pm", bufs=4, space="PSUM"))
        qtmp = ctx.enter_context(tc.tile_pool(name="qtmp", bufs=3))
        outp = ctx.enter_context(tc.tile_pool(name="outp", bufs=3))
        dram = ctx.enter_context(tc.tile_pool(name="dram", bufs=1, space="DRAM"))

        # constants
        h2_sb = singles.tile([P, P], f32)
        nc.sync.dma_start(out=h2_sb, in_=h2)
        bias_bc = singles.tile([P, m], f32)
        bias_bcast_ap = bass.AP(
            tensor=bias.tensor, offset=bias.offset,
            ap=[[0, P]] + list(bias.ap),
        )
        nc.sync.dma_start(out=bias_bc, in_=bias_bcast_ap)

        # resident quantized tensors
        wq = singles.tile([P, kc_n, m], bf16)
        xq = singles.tile([P, kc_n, tok], bf16)

        def rotation_pass(src, cols, ct, acc=None, r_b=None, dst=None):
            """Stream k-chunks of src [k, cols], rotate by h2 on PE.
            If acc: reduce abs-max of each psum tile into acc columns.
            Else: quantize with reciprocal-scale r_b into dst slices."""
            for kc in range(kc_n):
                ch = stream.tile([P, cols], f32, tag="chunk")
                nc.sync.dma_start(out=ch, in_=src[kc * P:(kc + 1) * P, :])
                for t in range(ct):
                    ps = pr.tile([P, nfree], f32)
                    nc.tensor.matmul(
                        ps, h2_sb, ch[:, t * nfree:(t + 1) * nfree],
                        start=True, stop=True,
                    )
                    if acc is not None:
                        idx = kc * ct + t
                        nc.vector.tensor_reduce(
                            out=acc[:, idx:idx + 1], in_=ps,
                            axis=mybir.AxisListType.X, op=mybir.AluOpType.max,
                            apply_absolute_value=True,
                        )
                    else:
                        q1 = qtmp.tile([P, nfree], f32, tag="q1")
                        nc.vector.tensor_scalar(
                            q1, ps, r_b, MAGIC,
                            mybir.AluOpType.mult, mybir.AluOpType.add,
                        )
                        nc.scalar.activation(
                            out=dst[:, kc, t * nfree:(t + 1) * nfree], in_=q1,
                            func=mybir.ActivationFunctionType.Copy,
                            bias=-MAGIC, scale=1.0,
                        )

        def finish_scale(acc, ncols):
            """acc [P, ncols] partial abs-maxes -> per-partition-replicated
            local max [P,1]."""
            mx = singles.tile([P, 1], f32, tag=f"mx{id(acc)}", name="mx")
            nc.vector.tensor_reduce(
                out=mx, in_=acc[:, :ncols], axis=mybir.AxisListType.X,
                op=mybir.AluOpType.max,
            )
            mxr = singles.tile([P, 1], f32, tag=f"mxr{id(acc)}", name="mxr")
            nc.gpsimd.partition_all_reduce(mxr, mx, channels=P, reduce_op=ReduceOp.max)
            return mxr

        # ---- pass 1: rotation + absmax ----
        wacc = singles.tile([P, kc_n * ct_w], f32)
        rotation_pass(wT, m, ct_w, acc=wacc)
        wmax_b = finish_scale(wacc, kc_n * ct_w)

        xacc = singles.tile([P, kc_n * ct_x], f32)
        rotation_pass(xT, tok, ct_x, acc=xacc)
        xmax_loc = finish_scale(xacc, kc_n * ct_x)

        # cross-core max for x (w is replicated -> identical everywhere)
        cc_in = dram.tile([1, 1], f32)
        cc_out = dram.tile([1, 1], f32)
        nc.sync.dma_start(out=cc_in, in_=xmax_loc[0:1, :])
        nc.gpsimd.collective_compute(
            "AllReduce", mybir.AluOpType.max,
            replica_groups=[list(range(ncores))],
            ins=[cc_in.opt()], outs=[cc_out.opt()],
        )
        xmax_b = singles.tile([P, 1], f32)
        nc.sync.dma_start(out=xmax_b, in_=cc_out.to_broadcast([P, 1]))

        # scales: s = max/127 (via mult by fp32 1/127: <=1 ulp vs divide,
        # flip probability negligible), r = 1/s via DVE iterative divide
        inv127 = float(np.float32(1.0) / np.float32(127.0))

        def srecip(mx_b, nm):
            s_b = singles.tile([P, 1], f32, tag=f"s{nm}", name="s_b")
            nc.vector.tensor_scalar_mul(s_b, mx_b, inv127)
            r_b = singles.tile([P, 1], f32, tag=f"r{nm}", name="r_b")
            nc.vector.reciprocal(r_b, s_b)
            return s_b, r_b

        sw_b, rw_b = srecip(wmax_b, "w")
        sx_b, rx_b = srecip(xmax_b, "x")

        # alpha = sx*sw/64  (the /64 undoes the two omitted 1/8 factors, exact)
        alpha_b = singles.tile([P, 1], f32)
        nc.vector.tensor_mul(alpha_b, sx_b, sw_b)
        nc.vector.tensor_scalar_mul(alpha_b, alpha_b, 0.015625)

        # ---- pass 2: rotation + quantize ----
        rotation_pass(wT, m, ct_w, r_b=rw_b, dst=wq)
        rotation_pass(xT, tok, ct_x, r_b=rx_b, dst=xq)

        # ---- main GEMM + fused dequant epilogue ----
        for nt in range(nt_n):
            for mt in range(mt_n):
                ps = pm.tile([P, nfree], f32)
                for kc in range(kc_n):
                    nc.tensor.matmul(
                        ps,
                        xq[:, kc, nt * P:(nt + 1) * P],
                        wq[:, kc, mt * nfree:(mt + 1) * nfree],
                        start=(kc == 0), stop=(kc == kc_n - 1),
                    )
                ob = outp.tile([P, nfree], f32)
                nc.vector.scalar_tensor_tensor(
                    out=ob, in0=ps, scalar=alpha_b,
                    in1=bias_bc[:, mt * nfree:(mt + 1) * nfree],
                    op0=mybir.AluOpType.mult, op1=mybir.AluOpType.add,
                )
                nc.sync.dma_start(
                    out=out[nt * P:(nt + 1) * P, mt * nfree:(mt + 1) * nfree],
                    in_=ob,
                )


def build(tok, k, m, ncores, rot_dtype="float32"):
    import concourse.mybir as mybir
    import concourse.tile as tile
    from concourse import bacc

    f32 = mybir.dt.float32
    nc = bacc.Bacc("TRN2", target_bir_lowering=False, debug=False,
                   num_devices=ncores)
    xT = nc.dram_tensor("xt", [k, tok], f32, kind="ExternalInput").ap()
    wT = nc.dram_tensor("wt", [k, m], f32, kind="ExternalInput").ap()
    bias = nc.dram_tensor("bias", [m], f32, kind="ExternalInput").ap()
    h2 = nc.dram_tensor("h2", [P, P], f32, kind="ExternalInput").ap()
    out = nc.dram_tensor("out", [tok, m], f32, kind="ExternalOutput").ap()

    with tile.TileContext(nc) as tc:
        _emit(tc, nc, xT, wT, bias, h2, out, tok, k, m, ncores, rot_dtype)
    nc.compile()
    return nc


LAST_RESULT = None


def kernel(x, weight, bias, hadamard_matrix):
    global LAST_RESULT
    from concourse.bass_utils import run_bass_kernel_spmd

    x = np.asarray(x, dtype=np.float32)
    weight = np.asarray(weight, dtype=np.float32)
    bias = np.asarray(bias, dtype=np.float32)
    h = np.asarray(hadamard_matrix, dtype=np.float32)

    in_shape = x.shape
    k = in_shape[-1]
    m = weight.shape[0]
    x2 = x.reshape(-1, k)
    ntok = x2.shape[0]
    tok = ntok // NCORES

    # host marshalling: block-diagonal H (2 blocks of 64), transposed layouts
    h2 = np.zeros((P, P), dtype=np.float32)
    h2[:BLOCK, :BLOCK] = h
    h2[BLOCK:, BLOCK:] = h
    wT = np.ascontiguousarray(weight.T)

    key = (tok, k, m, NCORES)
    if key not in _CACHE:
        _CACHE[key] = build(tok, k, m, NCORES)
    nc = _CACHE[key]

    in_maps = []
    for c in range(NCORES):
        xTc = np.ascontiguousarray(x2[c * tok:(c + 1) * tok].T)
        in_maps.append({"xt": xTc, "wt": wT, "bias": bias, "h2": h2})

    trace = os.environ.get("KERNEL_TRACE", "0") == "1"
    res = run_bass_kernel_spmd(
        nc, in_maps, core_ids=list(range(NCORES)), trace=trace,
    )
    LAST_RESULT = res
    out = np.concatenate([r["out"] for r in res.results], axis=0)
    return out.reshape(in_shape[:-1] + (m,))


# revision 10
# speedup vs baseline: 1.0716x; 1.0716x over previous
"""HadamardQuantLinear Trainium2 kernel.

Computes, for x:[B,S,K] f32, weight:[M,K] f32, bias:[M] f32, H:[64,64] f32:
    xb = blockhad(x, H/8); wb = blockhad(w, H/8)            (64-blocks on K)
    q, s = per-tensor symmetric int8 quant of xb and wb
    out = (xq @ wq.T) * (sx*sw) + bias

Sharding: data-parallel over tokens (B*S) across 8 cores; weight/bias/H
replicated. Host marshalling only: token-sharding, pre-transposed layouts
(K on partitions), and a 128x128 block-diagonal copy of H (pure data
movement; the 1/8 normalization folds exactly into the quant scales as a
power of two, so device-side math is bit-equivalent to the reference).

Device per core:
  pass1: rotate (PE, fp32) streaming xT/wT k-chunks, track |.|max     -> scales
         (x's max is AllReduce'd across the 8 cores; max is exact)
  pass2: rotate again, quantize with round-half-even (magic-number) to bf16
  main:  bf16 GEMM (values are small ints -> exact), fused dequant+bias
"""

import os

import numpy as np

NCORES = 8
P = 128
BLOCK = 64
MAGIC = 12582912.0  # 1.5 * 2**23: adding forces RNE to integer in fp32

_CACHE = {}


def _emit(tc, nc, xT, wT, bias, h2, out, tok, k, m, ncores, rot_dtype):
    import concourse.bass as bass
    import concourse.mybir as mybir
    from concourse.bass_isa import ReduceOp
    from contextlib import ExitStack

    f32 = mybir.dt.float32
    bf16 = mybir.dt.bfloat16

    kc_n = k // P            # number of 128-row k-chunks
    nfree = 512              # psum free dim
    ct_x = tok // nfree      # rotation column tiles per chunk (x)
    ct_w = m // nfree        # rotation column tiles per chunk (w)
    nt_n = tok // P          # output token tiles
    mt_n = m // nfree        # output feature tiles

    ctx = ExitStack()
    with ctx:
        singles = ctx.enter_context(tc.tile_pool(name="singles", bufs=1))
        stream = ctx.enter_context(tc.tile_pool(name="stream", bufs=4))
        pr = ctx.enter_context(tc.tile_pool(name="pr", bufs=2, space="PSUM"))
        pm = ctx.enter_context(tc.tile_pool(name="pm", bufs=2, space="PSUM"))
        qtmp = ctx.enter_context(tc.tile_pool(name="qtmp", bufs=2))
        outp = ctx.enter_context(tc.tile_pool(name="outp", bufs=2))
        dram = ctx.enter_context(tc.tile_pool(name="dram", bufs=1, space="DRAM"))

        # constants
        h2_sb = singles.tile([P, P], f32)
        nc.sync.dma_start(out=h2_sb, in_=h2)
        bias_bc = singles.tile([P, m], f32)
        bias_bcast_ap = bass.AP(
            tensor=bias.tensor, offset=bias.offset,
            ap=[[0, P]] + list(bias.ap),
        )
        nc.sync.dma_start(out=bias_bc, in_=bias_bcast_ap)

        # resident quantized tensors
        wq = singles.tile([P, kc_n, m], bf16)
        xq = singles.tile([P, kc_n, tok], bf16)

        def rotation_pass(src, cols, ct, acc=None, r_b=None, dst=None):
            """Stream k-chunks of src [k, cols], rotate by h2 on PE.
            Two 512-wide matmuls land in one 1024-wide (2-bank) psum tile so
            the DVE consumer (abs-max reduce or quantize) runs once per pair.
            If acc: reduce abs-max into acc columns; else quantize with
            reciprocal-scale r_b into dst slices."""
            for kc in range(kc_n):
                ch = stream.tile([P, cols], f32, tag="chunk")
                nc.sync.dma_start(out=ch, in_=src[kc * P:(kc + 1) * P, :])
                for t2 in range(ct // 2):
                    ps = pr.tile([P, 2 * nfree], f32)
                    for h in range(2):
                        t = 2 * t2 + h
                        nc.tensor.matmul(
                            ps[:, h * nfree:(h + 1) * nfree], h2_sb,
                            ch[:, t * nfree:(t + 1) * nfree],
                            start=True, stop=True,
                        )
                    if acc is not None:
                        idx = kc * (ct // 2) + t2
                        nc.vector.tensor_reduce(
                            out=acc[:, idx:idx + 1], in_=ps,
                            axis=mybir.AxisListType.X, op=mybir.AluOpType.max,
                            apply_absolute_value=True,
                        )
                    else:
                        q1 = qtmp.tile([P, 2 * nfree], f32, tag="q1")
                        nc.vector.tensor_scalar(
                            q1, ps, r_b, MAGIC,
                            mybir.AluOpType.mult, mybir.AluOpType.add,
                        )
                        nc.scalar.activation(
                            out=dst[:, kc, 2 * t2 * nfree:2 * (t2 + 1) * nfree],
                            in_=q1,
                            func=mybir.ActivationFunctionType.Copy,
                            bias=-MAGIC, scale=1.0,
                        )

        def finish_scale(acc, ncols):
            """acc [P, ncols] partial abs-maxes -> per-partition-replicated
            local max [P,1]."""
            mx = singles.tile([P, 1], f32, tag=f"mx{id(acc)}", name="mx")
            nc.vector.tensor_reduce(
                out=mx, in_=acc[:, :ncols], axis=mybir.AxisListType.X,
                op=mybir.AluOpType.max,
            )
            mxr = singles.tile([P, 1], f32, tag=f"mxr{id(acc)}", name="mxr")
            nc.gpsimd.partition_all_reduce(mxr, mx, channels=P, reduce_op=ReduceOp.max)
            return mxr

        inv127 = float(np.float32(1.0) / np.float32(127.0))

        def srecip(mx_b, nm):
            # s = max/127 (mult by fp32 1/127: <=1 ulp vs divide), r = 1/s
            s_b = singles.tile([P, 1], f32, tag=f"s{nm}", name="s_b")
            nc.vector.tensor_scalar_mul(s_b, mx_b, inv127)
            r_b = singles.tile([P, 1], f32, tag=f"r{nm}", name="r_b")
            nc.vector.reciprocal(r_b, s_b)
            return s_b, r_b

        # ---- X pass 1 first: its collective overlaps the whole W side ----
        xacc = singles.tile([P, kc_n * (ct_x // 2)], f32)
        rotation_pass(xT, tok, ct_x, acc=xacc)
        xmax_loc = finish_scale(xacc, kc_n * (ct_x // 2))

        # cross-core max for x (w is replicated -> identical everywhere)
        cc_in = dram.tile([1, 1], f32)
        cc_out = dram.tile([1, 1], f32)
        nc.sync.dma_start(out=cc_in, in_=xmax_loc[0:1, :])
        nc.gpsimd.collective_compute(
            "AllReduce", mybir.AluOpType.max,
            replica_groups=[list(range(ncores))],
            ins=[cc_in.opt()], outs=[cc_out.opt()],
        )

        # ---- W side (independent of the collective) ----
        wacc = singles.tile([P, kc_n * (ct_w // 2)], f32)
        rotation_pass(wT, m, ct_w, acc=wacc)
        wmax_b = finish_scale(wacc, kc_n * (ct_w // 2))
        sw_b, rw_b = srecip(wmax_b, "w")
        rotation_pass(wT, m, ct_w, r_b=rw_b, dst=wq)

        # ---- X scale (waits on collective) + pass 2 ----
        xmax_b = singles.tile([P, 1], f32)
        nc.sync.dma_start(out=xmax_b, in_=cc_out.to_broadcast([P, 1]))
        sx_b, rx_b = srecip(xmax_b, "x")

        # alpha = sx*sw/64  (the /64 undoes the two omitted 1/8 factors, exact)
        alpha_b = singles.tile([P, 1], f32)
        nc.vector.tensor_mul(alpha_b, sx_b, sw_b)
        nc.vector.tensor_scalar_mul(alpha_b, alpha_b, 0.015625)

        rotation_pass(xT, tok, ct_x, r_b=rx_b, dst=xq)

        # ---- main GEMM + fused dequant epilogue ----
        # two 512-wide accumulation groups per 1024-wide (2-bank) psum tile;
        # one epilogue + one out-DMA per 1024 columns
        for nt in range(nt_n):
            for mt2 in range(mt_n // 2):
                ps = pm.tile([P, 2 * nfree], f32)
                for h in range(2):
                    mt = 2 * mt2 + h
                    for kc in range(kc_n):
                        nc.tensor.matmul(
                            ps[:, h * nfree:(h + 1) * nfree],
                            xq[:, kc, nt * P:(nt + 1) * P],
                            wq[:, kc, mt * nfree:(mt + 1) * nfree],
                            start=(kc == 0), stop=(kc == kc_n - 1),
                        )
                ob = outp.tile([P, 2 * nfree], f32)
                nc.vector.scalar_tensor_tensor(
                    out=ob, in0=ps, scalar=alpha_b,
                    in1=bias_bc[:, 2 * mt2 * nfree:2 * (mt2 + 1) * nfree],
                    op0=mybir.AluOpType.mult, op1=mybir.AluOpType.add,
                )
                nc.sync.dma_start(
                    out=out[nt * P:(nt + 1) * P,
                            2 * mt2 * nfree:2 * (mt2 + 1) * nfree],
                    in_=ob,
                )


def build(tok, k, m, ncores, rot_dtype="float32"):
    import concourse.mybir as mybir
    import concourse.tile as tile
    from concourse import bacc

    f32 = mybir.dt.float32
    nc = bacc.Bacc("TRN2", target_bir_lowering=False, debug=False,
                   num_devices=ncores)
    xT = nc.dram_tensor("xt", [k, tok], f32, kind="ExternalInput").ap()
    wT = nc.dram_tensor("wt", [k, m], f32, kind="ExternalInput").ap()
    bias = nc.dram_tensor("bias", [m], f32, kind="ExternalInput").ap()
    h2 = nc.dram_tensor("h2", [P, P], f32, kind="ExternalInput").ap()
    out = nc.dram_tensor("out", [tok, m], f32, kind="ExternalOutput").ap()

    with tile.TileContext(nc) as tc:
        _emit(tc, nc, xT, wT, bias, h2, out, tok, k, m, ncores, rot_dtype)
    nc.compile()
    return nc


LAST_RESULT = None


def kernel(x, weight, bias, hadamard_matrix):
    global LAST_RESULT
    from concourse.bass_utils import run_bass_kernel_spmd

    x = np.asarray(x, dtype=np.float32)
    weight = np.asarray(weight, dtype=np.float32)
    bias = np.asarray(bias, dtype=np.float32)
    h = np.asarray(hadamard_matrix, dtype=np.float32)

    in_shape = x.shape
    k = in_shape[-1]
    m = weight.shape[0]
    x2 = x.reshape(-1, k)
    ntok = x2.shape[0]
    tok = ntok // NCORES

    # host marshalling: block-diagonal H (2 blocks of 64), transposed layouts
    h2 = np.zeros((P, P), dtype=np.float32)
    h2[:BLOCK, :BLOCK] = h
    h2[BLOCK:, BLOCK:] = h
    wT = np.ascontiguousarray(weight.T)

    key = (tok, k, m, NCORES)
    if key not in _CACHE:
        _CACHE[key] = build(tok, k, m, NCORES)
    nc = _CACHE[key]

    in_maps = []
    for c in range(NCORES):
        xTc = np.ascontiguousarray(x2[c * tok:(c + 1) * tok].T)
        in_maps.append({"xt": xTc, "wt": wT, "bias": bias, "h2": h2})

    trace = os.environ.get("KERNEL_TRACE", "0") == "1"
    res = run_bass_kernel_spmd(
        nc, in_maps, core_ids=list(range(NCORES)), trace=trace,
    )
    LAST_RESULT = res
    out = np.concatenate([r["out"] for r in res.results], axis=0)
    return out.reshape(in_shape[:-1] + (m,))
